# revision 24
# baseline (speedup 1.0000x reference)
"""Trainium2 Bass kernel for nn_Baseline_64080912056948 (VAE encoder + GRU-cell decoder).

Strategy (8 NeuronCores, pure SPMD, no collectives):
  - The encoder (2x Linear+ReLU+BatchNorm, Linear -> mu/logvar, z = mu+eps*std)
    couples the whole batch through BatchNorm training-mode stats, but it is
    tiny (~50 MFLOP). Every core computes it redundantly over the full batch.
  - Each core's inputs are batch-ROTATED by c*128 rows. BN stats and the KL
    mean are permutation-invariant, so the encoder output of core c is the
    full-batch result in rotated order; every core then decodes batch columns
    0:128 = its own shard. Same program on all cores, no partition-id needed.
  - The heavy part (GRU cell over [B, T-1, L] + skinny output matmul) is
    sharded by batch: 128 rows/core.
  - KL reduces on-chip to per-feature partials [128,1]; the host finishes the
    scalar (cross-partition sum of 128 values).

Layout: feature-major (features/L on partitions, batch/time on free dim) so
BatchNorm batch-reductions run along the free dim and all per-feature or
per-batch-row constants are per-partition scalars for fused tensor_scalar /
activation ops.

Sync-width constraint: walrus allows only ONE semaphore wait on a Matmult
(S3_LW) and about two on other instructions. Therefore every matmul operand is
produced by DVE (post-DMA staging copies) and every PSUM tile has exactly one
DVE consumer, so each matmul's waits merge into a single DVE semaphore.
"""

import json

import numpy as np

import concourse.bass as bass
import concourse.tile as tile
from concourse import mybir
from concourse import bass_utils
import concourse.bass2jax as bass2jax

# ---------------------------------------------------------------------------
# This walrus build accepts at most ONE semaphore wait per instruction.
# Tile emits several. Rewrite the BIR before compilation: hoist all but one
# wait of each instruction onto same-engine single-wait NoOps placed directly
# before it (same queue, program order => semantics preserved).
_nop_ctr = [0]


def _split_multi_waits(bir_json):
    if isinstance(bir_json, bytes):
        d = json.loads(bir_json.decode())
    else:
        d = json.loads(bir_json)

    def fix_blocks(obj):
        if isinstance(obj, dict):
            if "instructions" in obj and isinstance(obj["instructions"], list):
                out = []
                for ins in obj["instructions"]:
                    si = ins.get("sync_info") if isinstance(ins, dict) else None
                    ow = (si or {}).get("on_wait") or []
                    if len(ow) > 1:
                        for w in ow[:-1]:
                            _nop_ctr[0] += 1
                            out.append({
                                "debug": ins.get("debug", 0),
                                "engine": ins["engine"],
                                "ins": [],
                                "name": f"{ins['name']}-sw{_nop_ctr[0]}",
                                "opcode": "NoOp",
                                "outs": [],
                                "sync_info": {"on_update": [], "on_wait": [w]},
                            })
                        si["on_wait"] = [ow[-1]]
                    out.append(ins)
                obj["instructions"] = out
            for v in obj.values():
                fix_blocks(v)
        elif isinstance(obj, list):
            for v in obj:
                fix_blocks(v)

    fix_blocks(d.get("functions"))
    return json.dumps(d).encode()


_orig_compile_bir_kernel = bass_utils.compile_bir_kernel


def _patched_compile_bir_kernel(bir_json, tmpdir, neff_name="file.neff"):
    return _orig_compile_bir_kernel(_split_multi_waits(bir_json), tmpdir, neff_name)


bass_utils.compile_bir_kernel = _patched_compile_bir_kernel
bass2jax.compile_bir_kernel = _patched_compile_bir_kernel
# ---------------------------------------------------------------------------

F32 = mybir.dt.float32
AF = mybir.ActivationFunctionType
ALU = mybir.AluOpType
AX = mybir.AxisListType

B, T, L, IN, H1 = 1024, 256, 128, 128, 100
TD = T - 1          # 255 decode steps
NC = 8              # cores
BS = B // NC        # 128 batch rows per core
BN_EPS = 1e-5
GRP = 8             # decode batch rows per DMA/staging group

_cache = {}


def _build_bass():
    nc = bass.Bass("TRN2", target_bir_lowering=False)

    def din(name, shape):
        return nc.dram_tensor(name, shape, F32, kind="ExternalInput").ap()

    xinT = din("xinT", [IN, B])        # x[:, 0, :128].T, batch-rotated per core
    epsT = din("epsT", [L, B])         # eps.T, batch-rotated per core
    xs = din("xs", [1, BS * TD])       # xs shard, flat; broadcast via stride-0 DMA read
    w1t = din("w1t", [IN, H1])
    w2t = din("w2t", [H1, H1])
    w3t = din("w3t", [H1, 2 * L])
    whht = din("whht", [L, 3 * L])
    wft = din("wft", [L, 2])
    b1c = din("b1c", [H1, 1])
    g1c = din("g1c", [H1, 1])
    be1c = din("be1c", [H1, 1])
    b2c = din("b2c", [H1, 1])
    g2c = din("g2c", [H1, 1])
    be2c = din("be2c", [H1, 1])
    b3mu = din("b3mu", [L, 1])
    b3lv = din("b3lv", [L, 1])
    b3lvh = din("b3lvh", [L, 1])       # 0.5 * b3[L:]
    wihc = din("wihc", [L, 3])         # Wih[:,0] as 3 per-partition columns r,z,n
    bbc = din("bbc", [L, 3])           # col0=(bih+bhh)_r col1=(bih+bhh)_z col2=bhh_n
    bihn = din("bihn", [L, 1])         # bih_n

    preds_out = nc.dram_tensor("preds", [2, BS * TD], F32, kind="ExternalOutput").ap()
    zt_out = nc.dram_tensor("zt", [L, B], F32, kind="ExternalOutput").ap()
    klp_out = nc.dram_tensor("klp", [L, 1], F32, kind="ExternalOutput").ap()

    with tile.TileContext(nc) as tc:
        with (
            tc.tile_pool(name="big", bufs=1) as big,
            tc.tile_pool(name="wpool", bufs=1) as wp,
            tc.tile_pool(name="small", bufs=4) as sm,
            tc.tile_pool(name="dec", bufs=4) as dec,
            tc.tile_pool(name="ps", bufs=2, space="PSUM") as ps,
            tc.tile_pool(name="pso", bufs=4, space="PSUM") as pso,
        ):
            # ---- load + DVE staging copy (keeps DMA sems off matmuls) ----
            def load(ap_dram, shape, stage=True):
                t = wp.tile(shape, F32, tag=ap_dram.tensor.name)
                nc.gpsimd.dma_start(out=t, in_=ap_dram)
                if not stage:
                    return t
                t2 = wp.tile(shape, F32, tag=ap_dram.tensor.name + "_c")
                nc.vector.tensor_copy(t2, t)
                return t2

            xinT_s = load(xinT, [IN, B])
            epsT_s = load(epsT, [L, B], stage=False)
            w1t_s = load(w1t, [IN, H1])
            w2t_s = load(w2t, [H1, H1])
            w3t_s = load(w3t, [H1, 2 * L])
            whht_s = load(whht, [L, 3 * L])
            wft_s = load(wft, [L, 2])
            b1c_s = load(b1c, [H1, 1])
            g1c_s = load(g1c, [H1, 1])
            be1c_s = load(be1c, [H1, 1])
            b2c_s = load(b2c, [H1, 1])
            g2c_s = load(g2c, [H1, 1])
            be2c_s = load(be2c, [H1, 1])
            b3mu_s = load(b3mu, [L, 1])
            b3lv_s = load(b3lv, [L, 1])
            b3lvh_s = load(b3lvh, [L, 1])
            wihc_s = load(wihc, [L, 3])
            bbc_s = load(bbc, [L, 3])
            bihn_s = load(bihn, [L, 1])

            NCH = 2          # encoder free-dim chunks of 512
            CW = B // NCH

            zcol = sm.tile([128, 1], F32, tag="zcol")
            nc.vector.memset(zcol, 0.0)
            epscol = sm.tile([128, 1], F32, tag="epscol")
            nc.vector.memset(epscol, BN_EPS)

            # ---------- encoder layer: h = BN(relu(W @ in + b)) ----------
            def mlp_bn_layer(inT_s, wt_s, p_in, bcol, gcol, becol, li):
                hT = big.tile([H1, B], F32, tag=f"h{li}")
                for c in range(NCH):
                    pre = ps.tile([H1, CW], F32, tag="encmm")
                    nc.tensor.matmul(pre, wt_s[:p_in, :], inT_s[:p_in, c * CW:(c + 1) * CW],
                                     start=True, stop=True)
                    # relu(pre + b) fused on DVE: (pre + b) max 0
                    nc.vector.tensor_scalar(hT[:, c * CW:(c + 1) * CW], pre,
                                            bcol, 0.0, op0=ALU.add, op1=ALU.max)
                stats = sm.tile([H1, NCH, 6], F32, tag="bnstats")
                for c in range(NCH):
                    nc.vector.bn_stats(stats[:, c, :], hT[:, c * CW:(c + 1) * CW])
                mv = sm.tile([H1, 2], F32, tag="bnmv")
                nc.vector.bn_aggr(mv, stats)
                rstd = sm.tile([H1, 1], F32, tag="rstd")
                nc.scalar.activation(rstd, mv[:, 1:2], AF.Sqrt, bias=epscol[:H1], scale=1.0)
                nc.vector.reciprocal(rstd, rstd)
                sc = sm.tile([H1, 1], F32, tag="bnsc")
                nc.vector.tensor_mul(sc, gcol, rstd)          # gamma / std
                sh = sm.tile([H1, 1], F32, tag="bnsh")
                nc.vector.tensor_mul(sh, mv[:, 0:1], sc)      # mean * gamma/std
                nc.vector.tensor_sub(sh, becol, sh)           # beta - mean*gamma/std
                outT = big.tile([H1, B], F32, tag=f"bn{li}")
                nc.vector.tensor_scalar(outT, hT, sc, sh, op0=ALU.mult, op1=ALU.add)
                return outT

            bn1T = mlp_bn_layer(xinT_s, w1t_s, IN, b1c_s, g1c_s, be1c_s, 1)
            bn2T = mlp_bn_layer(bn1T, w2t_s, H1, b2c_s, g2c_s, be2c_s, 2)

            # ---------- mu / std / logvar / z / kl ----------
            muT = big.tile([L, B], F32)
            stdT = big.tile([L, B], F32)
            lvT = big.tile([L, B], F32)
            zT = big.tile([L, B], F32)
            sq = big.tile([L, CW], F32)     # scratch for Square
            acc = sm.tile([L, 6], F32, tag="klacc")
            for c in range(NCH):
                cs = slice(c * CW, (c + 1) * CW)
                pmu = ps.tile([L, CW], F32, tag="encmm")
                nc.tensor.matmul(pmu, w3t_s[:, 0:L], bn2T[:, cs], start=True, stop=True)
                nc.vector.tensor_scalar(muT[:, cs], pmu, b3mu_s, None, op0=ALU.add)
                plv = ps.tile([L, CW], F32, tag="encmm")
                nc.tensor.matmul(plv, w3t_s[:, L:2 * L], bn2T[:, cs], start=True, stop=True)
                lvpre = big.tile([L, CW], F32, tag="lvpre")
                nc.vector.tensor_copy(lvpre, plv)
                nc.scalar.activation(stdT[:, cs], lvpre, AF.Exp, bias=b3lvh_s, scale=0.5)
                nc.vector.tensor_scalar(lvT[:, cs], lvpre, b3lv_s, None, op0=ALU.add)
                nc.scalar.activation(sq, muT[:, cs], AF.Square, bias=zcol[:L],
                                     accum_out=acc[:, c:c + 1])
                nc.scalar.activation(sq, stdT[:, cs], AF.Square, bias=zcol[:L],
                                     accum_out=acc[:, 2 + c:3 + c])
                nc.vector.reduce_sum(acc[:, 4 + c:5 + c], lvT[:, cs], axis=AX.X)
                nc.vector.tensor_mul(zT[:, cs], epsT_s[:, cs], stdT[:, cs])
                nc.vector.tensor_add(zT[:, cs], muT[:, cs], zT[:, cs])
            klp = sm.tile([L, 1], F32, tag="klp")
            nc.vector.tensor_sub(klp, acc[:, 4:5], acc[:, 0:1])
            nc.vector.tensor_sub(klp, klp, acc[:, 2:3])
            nc.vector.tensor_add(klp, klp, acc[:, 5:6])
            nc.vector.tensor_sub(klp, klp, acc[:, 1:2])
            nc.vector.tensor_sub(klp, klp, acc[:, 3:4])
            nc.gpsimd.dma_start(out=klp_out, in_=klp)
            nc.gpsimd.dma_start(out=zt_out, in_=zT)

            # ---------- decode prep: c_g = Whh_g @ z_shard + bias ----------
            cgate = big.tile([L, 3 * L], F32)     # [cr | cz | cn] per shard batch col
            for g in range(3):
                pg_full = ps.tile([L, CW], F32, tag="encmm")
                pg = pg_full[:, :BS]
                nc.tensor.matmul(pg, whht_s[:, g * L:(g + 1) * L], zT[:, 0:BS],
                                 start=True, stop=True)
                nc.vector.tensor_scalar(cgate[:, g * L:(g + 1) * L], pg,
                                        bbc_s[:, g:g + 1], None, op0=ALU.add)

            # ---------- per-batch-row GRU cell + output matmul ----------
            for b in range(BS):
                bcol = slice(b, b + 1)
                g = b % GRP
                if g == 0:
                    xbc = dec.tile([L, GRP * TD], F32, tag="xbc")
                    xs_bc = bass.AP(tensor=xs.tensor, offset=b * TD,
                                    ap=[[0, L], [1, GRP * TD]])
                    nc.gpsimd.dma_start(out=xbc, in_=xs_bc)
                    obs = dec.tile([2, GRP * TD], F32, tag="obs")
                    # t1 = wih_n*x + bih_n is batch-row-independent: one wide op
                    t1g = dec.tile([L, GRP * TD], F32, tag="t1g")
                    nc.vector.tensor_scalar(t1g, xbc, wihc_s[:, 2:3], bihn_s,
                                            op0=ALU.mult, op1=ALU.add)
                xv = xbc[:, g * TD:(g + 1) * TD]
                r = dec.tile([L, TD], F32, tag="r")
                nc.scalar.activation(r, xv, AF.Sigmoid,
                                     bias=cgate[:, 0 * L + b:0 * L + b + 1],
                                     scale=wihc_s[:, 0:1])
                zg = dec.tile([L, TD], F32, tag="zg")
                nc.scalar.activation(zg, xv, AF.Sigmoid,
                                     bias=cgate[:, 1 * L + b:1 * L + b + 1],
                                     scale=wihc_s[:, 1:2])
                t2 = dec.tile([L, TD], F32, tag="t2")
                # t2 = r * cn_b + t1 in one fused DVE op
                nc.vector.scalar_tensor_tensor(
                    t2, r, cgate[:, 2 * L + b:2 * L + b + 1],
                    t1g[:, g * TD:(g + 1) * TD],
                    op0=ALU.mult, op1=ALU.add)
                n = dec.tile([L, TD], F32, tag="n")
                nc.scalar.activation(n, t2, AF.Tanh, bias=zcol[:L])
                # d = z_b - n ; h = n + zg*d ; hr = relu(h)
                d = dec.tile([L, TD], F32, tag="d")
                nc.vector.tensor_scalar(d, n, -1.0, zT[:, bcol], op0=ALU.mult, op1=ALU.add)
                nc.vector.tensor_mul(d, zg, d)
                nc.vector.tensor_add(d, n, d)
                hr = dec.tile([L, TD], F32, tag="hr")
                nc.vector.tensor_scalar(hr, d, 0.0, None, op0=ALU.max)
                if b % 2 == 0:
                    obp = pso.tile([2, 2 * TD], F32, tag="ob")
                nc.tensor.matmul(obp[:, (b % 2) * TD:(b % 2 + 1) * TD],
                                 wft_s, hr, start=True, stop=True)
                if b % 2 == 1:
                    nc.vector.tensor_copy(obs[:, (g - 1) * TD:(g + 1) * TD], obp)
                if g == GRP - 1:
                    nc.gpsimd.dma_start(
                        out=preds_out[:, (b - GRP + 1) * TD:(b + 1) * TD], in_=obs)

    return nc


def _prep_inputs(x, eps, W1, b1, g1, beta1, W2, b2, g2, beta2, W3, b3,
                 Wih, Whh, bih, bhh, Wf, bf):
    x2 = np.ascontiguousarray(x[:, 0, :])                   # [B, 256]
    xin = np.ascontiguousarray(x2[:, :IN])                  # [B, 128]
    wih = np.ascontiguousarray(Wih[:, 0])                   # [3L]
    bb = bih + bhh
    common = {
        "w1t": np.ascontiguousarray(W1.T),
        "w2t": np.ascontiguousarray(W2.T),
        "w3t": np.ascontiguousarray(W3.T),
        "whht": np.ascontiguousarray(Whh.T),
        "wft": np.ascontiguousarray(Wf.T),
        "b1c": b1.reshape(H1, 1).copy(),
        "g1c": g1.reshape(H1, 1).copy(),
        "be1c": beta1.reshape(H1, 1).copy(),
        "b2c": b2.reshape(H1, 1).copy(),
        "g2c": g2.reshape(H1, 1).copy(),
        "be2c": beta2.reshape(H1, 1).copy(),
        "b3mu": b3[:L].reshape(L, 1).copy(),
        "b3lv": b3[L:].reshape(L, 1).copy(),
        "b3lvh": (0.5 * b3[L:]).reshape(L, 1).copy(),
        "wihc": np.ascontiguousarray(wih.reshape(3, L).T),
        "bbc": np.ascontiguousarray(
            np.stack([bb[:L], bb[L:2 * L], bhh[2 * L:]], axis=1)),
        "bihn": bih[2 * L:].reshape(L, 1).copy(),
    }
    in_maps = []
    for c in range(NC):
        m = dict(common)
        m["xinT"] = np.ascontiguousarray(np.roll(xin, -c * BS, axis=0).T)
        m["epsT"] = np.ascontiguousarray(np.roll(eps, -c * BS, axis=0).T)
        m["xs"] = np.ascontiguousarray(
            x2[c * BS:(c + 1) * BS, :TD]).reshape(1, BS * TD)
        in_maps.append({k: np.asarray(v, np.float32) for k, v in m.items()})
    return in_maps


def run(trace=False, **inputs):
    inputs = {k: np.asarray(v, np.float32) for k, v in inputs.items()}
    in_maps = _prep_inputs(**inputs)
    if "nc" not in _cache:
        _cache["nc"] = _build_bass()
    res = bass_utils.run_bass_kernel_spmd(
        _cache["nc"], in_maps, core_ids=list(range(NC)), trace=trace)
    outs = res.results
    bf = inputs["bf"]
    preds = np.concatenate(
        [outs[c]["preds"].reshape(2, BS, TD).transpose(1, 0, 2) for c in range(NC)],
        axis=0)
    preds = preds + bf.reshape(1, 2, 1)
    predictions = preds[:, None]                            # [B, 1, 2, TD]
    klp = outs[0]["klp"].reshape(L)
    kl = np.float32(-0.5 * (L + float(klp.sum()) / B))
    z = np.ascontiguousarray(outs[0]["zt"].T)               # [B, L]
    return (predictions, kl, z), res


def kernel(**inputs):
    out, _ = run(trace=False, **inputs)
    return out


# revision 25
# speedup vs baseline: 1.1408x; 1.1408x over previous
"""Trainium2 Bass kernel for nn_Baseline_64080912056948 (VAE encoder + GRU-cell decoder).

Strategy (8 NeuronCores, pure SPMD, no collectives):
  - The encoder (2x Linear+ReLU+BatchNorm, Linear -> mu/logvar, z = mu+eps*std)
    couples the whole batch through BatchNorm training-mode stats, but it is
    tiny (~50 MFLOP). Every core computes it redundantly over the full batch.
  - Each core's inputs are batch-ROTATED by c*128 rows. BN stats and the KL
    mean are permutation-invariant, so the encoder output of core c is the
    full-batch result in rotated order; every core then decodes batch columns
    0:128 = its own shard. Same program on all cores, no partition-id needed.
  - The heavy part (GRU cell over [B, T-1, L] + skinny output matmul) is
    sharded by batch: 128 rows/core.
  - KL reduces on-chip to per-feature partials [128,1]; the host finishes the
    scalar (cross-partition sum of 128 values).

Layout: feature-major (features/L on partitions, batch/time on free dim) so
BatchNorm batch-reductions run along the free dim and all per-feature or
per-batch-row constants are per-partition scalars for fused tensor_scalar /
activation ops.

Sync-width constraint: walrus allows only ONE semaphore wait on a Matmult
(S3_LW) and about two on other instructions. Therefore every matmul operand is
produced by DVE (post-DMA staging copies) and every PSUM tile has exactly one
DVE consumer, so each matmul's waits merge into a single DVE semaphore.
"""

import json

import numpy as np

import concourse.bass as bass
import concourse.tile as tile
from concourse import mybir
from concourse import bass_utils
import concourse.bass2jax as bass2jax

# ---------------------------------------------------------------------------
# This walrus build accepts at most ONE semaphore wait per instruction.
# Tile emits several. Rewrite the BIR before compilation: hoist all but one
# wait of each instruction onto same-engine single-wait NoOps placed directly
# before it (same queue, program order => semantics preserved).
_nop_ctr = [0]


def _split_multi_waits(bir_json):
    if isinstance(bir_json, bytes):
        d = json.loads(bir_json.decode())
    else:
        d = json.loads(bir_json)

    def fix_blocks(obj):
        if isinstance(obj, dict):
            if "instructions" in obj and isinstance(obj["instructions"], list):
                out = []
                for ins in obj["instructions"]:
                    si = ins.get("sync_info") if isinstance(ins, dict) else None
                    ow = (si or {}).get("on_wait") or []
                    if len(ow) > 1:
                        for w in ow[:-1]:
                            _nop_ctr[0] += 1
                            out.append({
                                "debug": ins.get("debug", 0),
                                "engine": ins["engine"],
                                "ins": [],
                                "name": f"{ins['name']}-sw{_nop_ctr[0]}",
                                "opcode": "NoOp",
                                "outs": [],
                                "sync_info": {"on_update": [], "on_wait": [w]},
                            })
                        si["on_wait"] = [ow[-1]]
                    out.append(ins)
                obj["instructions"] = out
            for v in obj.values():
                fix_blocks(v)
        elif isinstance(obj, list):
            for v in obj:
                fix_blocks(v)

    fix_blocks(d.get("functions"))
    return json.dumps(d).encode()


_orig_compile_bir_kernel = bass_utils.compile_bir_kernel


def _patched_compile_bir_kernel(bir_json, tmpdir, neff_name="file.neff"):
    return _orig_compile_bir_kernel(_split_multi_waits(bir_json), tmpdir, neff_name)


bass_utils.compile_bir_kernel = _patched_compile_bir_kernel
bass2jax.compile_bir_kernel = _patched_compile_bir_kernel
# ---------------------------------------------------------------------------

F32 = mybir.dt.float32
AF = mybir.ActivationFunctionType
ALU = mybir.AluOpType
AX = mybir.AxisListType

B, T, L, IN, H1 = 1024, 256, 128, 128, 100
TD = T - 1          # 255 decode steps
NC = 8              # cores
BS = B // NC        # 128 batch rows per core
BN_EPS = 1e-5
GRP = 8             # decode batch rows per DMA/staging group

_cache = {}


def _build_bass():
    nc = bass.Bass("TRN2", target_bir_lowering=False)

    def din(name, shape):
        return nc.dram_tensor(name, shape, F32, kind="ExternalInput").ap()

    xinT = din("xinT", [IN, B])        # x[:, 0, :128].T, batch-rotated per core
    epsT = din("epsT", [L, B])         # eps.T, batch-rotated per core
    xs = din("xs", [1, BS * TD])       # xs shard, flat; broadcast via stride-0 DMA read
    w1t = din("w1t", [IN, H1])
    w2t = din("w2t", [H1, H1])
    w3t = din("w3t", [H1, 2 * L])
    whht = din("whht", [L, 3 * L])
    wft = din("wft", [L, 2])
    b1c = din("b1c", [H1, 1])
    g1c = din("g1c", [H1, 1])
    be1c = din("be1c", [H1, 1])
    b2c = din("b2c", [H1, 1])
    g2c = din("g2c", [H1, 1])
    be2c = din("be2c", [H1, 1])
    b3mu = din("b3mu", [L, 1])
    b3lv = din("b3lv", [L, 1])
    b3lvh = din("b3lvh", [L, 1])       # 0.5 * b3[L:]
    wihc = din("wihc", [L, 3])         # Wih[:,0] as 3 per-partition columns r,z,n
    bbc = din("bbc", [L, 3])           # col0=(bih+bhh)_r col1=(bih+bhh)_z col2=bhh_n
    bihn = din("bihn", [L, 1])         # bih_n

    preds_out = nc.dram_tensor("preds", [2, BS * TD], F32, kind="ExternalOutput").ap()
    zt_out = nc.dram_tensor("zt", [L, B], F32, kind="ExternalOutput").ap()
    klp_out = nc.dram_tensor("klp", [L, 1], F32, kind="ExternalOutput").ap()

    with tile.TileContext(nc) as tc:
        with (
            tc.tile_pool(name="big", bufs=1) as big,
            tc.tile_pool(name="wpool", bufs=1) as wp,
            tc.tile_pool(name="small", bufs=4) as sm,
            tc.tile_pool(name="dec", bufs=3) as dec,
            tc.tile_pool(name="ps", bufs=2, space="PSUM") as ps,
            tc.tile_pool(name="pso", bufs=2, space="PSUM") as pso,
        ):
            # ---- load + DVE staging copy (keeps DMA sems off matmuls) ----
            def load(ap_dram, shape, stage=True):
                t = wp.tile(shape, F32, tag=ap_dram.tensor.name)
                nc.gpsimd.dma_start(out=t, in_=ap_dram)
                if not stage:
                    return t
                t2 = wp.tile(shape, F32, tag=ap_dram.tensor.name + "_c")
                nc.vector.tensor_copy(t2, t)
                return t2

            xinT_s = load(xinT, [IN, B])
            epsT_s = load(epsT, [L, B], stage=False)
            w1t_s = load(w1t, [IN, H1])
            w2t_s = load(w2t, [H1, H1])
            w3t_s = load(w3t, [H1, 2 * L])
            whht_s = load(whht, [L, 3 * L])
            wft_s = load(wft, [L, 2])
            b1c_s = load(b1c, [H1, 1])
            g1c_s = load(g1c, [H1, 1])
            be1c_s = load(be1c, [H1, 1])
            b2c_s = load(b2c, [H1, 1])
            g2c_s = load(g2c, [H1, 1])
            be2c_s = load(be2c, [H1, 1])
            b3mu_s = load(b3mu, [L, 1])
            b3lv_s = load(b3lv, [L, 1])
            b3lvh_s = load(b3lvh, [L, 1])
            wihc_s = load(wihc, [L, 3])
            bbc_s = load(bbc, [L, 3])
            bihn_s = load(bihn, [L, 1])

            NCH = 2          # encoder free-dim chunks of 512
            CW = B // NCH

            zcol = sm.tile([128, 1], F32, tag="zcol")
            nc.vector.memset(zcol, 0.0)
            epscol = sm.tile([128, 1], F32, tag="epscol")
            nc.vector.memset(epscol, BN_EPS)

            # ---------- encoder layer: h = BN(relu(W @ in + b)) ----------
            def mlp_bn_layer(inT_s, wt_s, p_in, bcol, gcol, becol, li):
                hT = big.tile([H1, B], F32, tag=f"h{li}")
                for c in range(NCH):
                    pre = ps.tile([H1, CW], F32, tag="encmm")
                    nc.tensor.matmul(pre, wt_s[:p_in, :], inT_s[:p_in, c * CW:(c + 1) * CW],
                                     start=True, stop=True)
                    # relu(pre + b) fused on DVE: (pre + b) max 0
                    nc.vector.tensor_scalar(hT[:, c * CW:(c + 1) * CW], pre,
                                            bcol, 0.0, op0=ALU.add, op1=ALU.max)
                stats = sm.tile([H1, NCH, 6], F32, tag="bnstats")
                for c in range(NCH):
                    nc.vector.bn_stats(stats[:, c, :], hT[:, c * CW:(c + 1) * CW])
                mv = sm.tile([H1, 2], F32, tag="bnmv")
                nc.vector.bn_aggr(mv, stats)
                rstd = sm.tile([H1, 1], F32, tag="rstd")
                nc.scalar.activation(rstd, mv[:, 1:2], AF.Sqrt, bias=epscol[:H1], scale=1.0)
                nc.vector.reciprocal(rstd, rstd)
                sc = sm.tile([H1, 1], F32, tag="bnsc")
                nc.vector.tensor_mul(sc, gcol, rstd)          # gamma / std
                sh = sm.tile([H1, 1], F32, tag="bnsh")
                nc.vector.tensor_mul(sh, mv[:, 0:1], sc)      # mean * gamma/std
                nc.vector.tensor_sub(sh, becol, sh)           # beta - mean*gamma/std
                outT = big.tile([H1, B], F32, tag=f"bn{li}")
                nc.vector.tensor_scalar(outT, hT, sc, sh, op0=ALU.mult, op1=ALU.add)
                return outT

            bn1T = mlp_bn_layer(xinT_s, w1t_s, IN, b1c_s, g1c_s, be1c_s, 1)
            bn2T = mlp_bn_layer(bn1T, w2t_s, H1, b2c_s, g2c_s, be2c_s, 2)

            # ---------- mu / std / logvar / z / kl ----------
            muT = big.tile([L, B], F32)
            stdT = big.tile([L, B], F32)
            lvT = big.tile([L, B], F32)
            zT = big.tile([L, B], F32)
            sq = big.tile([L, CW], F32)     # scratch for Square
            acc = sm.tile([L, 6], F32, tag="klacc")
            for c in range(NCH):
                cs = slice(c * CW, (c + 1) * CW)
                pmu = ps.tile([L, CW], F32, tag="encmm")
                nc.tensor.matmul(pmu, w3t_s[:, 0:L], bn2T[:, cs], start=True, stop=True)
                nc.vector.tensor_scalar(muT[:, cs], pmu, b3mu_s, None, op0=ALU.add)
                plv = ps.tile([L, CW], F32, tag="encmm")
                nc.tensor.matmul(plv, w3t_s[:, L:2 * L], bn2T[:, cs], start=True, stop=True)
                lvpre = big.tile([L, CW], F32, tag="lvpre")
                nc.vector.tensor_copy(lvpre, plv)
                nc.scalar.activation(stdT[:, cs], lvpre, AF.Exp, bias=b3lvh_s, scale=0.5)
                nc.vector.tensor_scalar(lvT[:, cs], lvpre, b3lv_s, None, op0=ALU.add)
                nc.scalar.activation(sq, muT[:, cs], AF.Square, bias=zcol[:L],
                                     accum_out=acc[:, c:c + 1])
                nc.scalar.activation(sq, stdT[:, cs], AF.Square, bias=zcol[:L],
                                     accum_out=acc[:, 2 + c:3 + c])
                nc.vector.reduce_sum(acc[:, 4 + c:5 + c], lvT[:, cs], axis=AX.X)
                nc.vector.tensor_mul(zT[:, cs], epsT_s[:, cs], stdT[:, cs])
                nc.vector.tensor_add(zT[:, cs], muT[:, cs], zT[:, cs])
            klp = sm.tile([L, 1], F32, tag="klp")
            nc.vector.tensor_sub(klp, acc[:, 4:5], acc[:, 0:1])
            nc.vector.tensor_sub(klp, klp, acc[:, 2:3])
            nc.vector.tensor_add(klp, klp, acc[:, 5:6])
            nc.vector.tensor_sub(klp, klp, acc[:, 1:2])
            nc.vector.tensor_sub(klp, klp, acc[:, 3:4])
            nc.gpsimd.dma_start(out=klp_out, in_=klp)
            nc.gpsimd.dma_start(out=zt_out, in_=zT)

            # ---------- decode prep: c_g = Whh_g @ z_shard + bias ----------
            cgate = big.tile([L, 3 * L], F32)     # [cr | cz | cn] per shard batch col
            for g in range(3):
                pg_full = ps.tile([L, CW], F32, tag="encmm")
                pg = pg_full[:, :BS]
                nc.tensor.matmul(pg, whht_s[:, g * L:(g + 1) * L], zT[:, 0:BS],
                                 start=True, stop=True)
                nc.vector.tensor_scalar(cgate[:, g * L:(g + 1) * L], pg,
                                        bbc_s[:, g:g + 1], None, op0=ALU.add)

            # ---------- per-batch-row GRU cell + output matmul ----------
            for b in range(BS):
                bcol = slice(b, b + 1)
                g = b % GRP
                if g == 0:
                    xbc = dec.tile([L, GRP * TD], F32, tag="xbc")
                    xs_bc = bass.AP(tensor=xs.tensor, offset=b * TD,
                                    ap=[[0, L], [1, GRP * TD]])
                    nc.gpsimd.dma_start(out=xbc, in_=xs_bc)
                    obs = dec.tile([2, GRP * TD], F32, tag="obs")
                    # t1 = wih_n*x + bih_n is batch-row-independent: one wide op
                    t1g = dec.tile([L, GRP * TD], F32, tag="t1g")
                    nc.vector.tensor_scalar(t1g, xbc, wihc_s[:, 2:3], bihn_s,
                                            op0=ALU.mult, op1=ALU.add)
                xv = xbc[:, g * TD:(g + 1) * TD]
                r = dec.tile([L, TD], F32, tag="r")
                nc.scalar.activation(r, xv, AF.Sigmoid,
                                     bias=cgate[:, 0 * L + b:0 * L + b + 1],
                                     scale=wihc_s[:, 0:1])
                zg = dec.tile([L, TD], F32, tag="zg")
                nc.scalar.activation(zg, xv, AF.Sigmoid,
                                     bias=cgate[:, 1 * L + b:1 * L + b + 1],
                                     scale=wihc_s[:, 1:2])
                t2 = dec.tile([L, TD], F32, tag="t2")
                # t2 = r * cn_b + t1 in one fused DVE op
                nc.vector.scalar_tensor_tensor(
                    t2, r, cgate[:, 2 * L + b:2 * L + b + 1],
                    t1g[:, g * TD:(g + 1) * TD],
                    op0=ALU.mult, op1=ALU.add)
                n = dec.tile([L, TD], F32, tag="n")
                nc.scalar.activation(n, t2, AF.Tanh, bias=zcol[:L])
                # d = z_b - n ; h = n + zg*d ; hr = relu(h)
                d = dec.tile([L, TD], F32, tag="d")
                nc.vector.tensor_scalar(d, n, -1.0, zT[:, bcol], op0=ALU.mult, op1=ALU.add)
                nc.vector.tensor_mul(d, zg, d)
                nc.vector.tensor_add(d, n, d)
                hr = dec.tile([L, TD], F32, tag="hr")
                nc.vector.tensor_scalar(hr, d, 0.0, None, op0=ALU.max)
                if b % 2 == 0:
                    obp = pso.tile([2, 2 * TD], F32, tag="ob")
                nc.tensor.matmul(obp[:, (b % 2) * TD:(b % 2 + 1) * TD],
                                 wft_s, hr, start=True, stop=True)
                if b % 2 == 1:
                    nc.vector.tensor_copy(obs[:, (g - 1) * TD:(g + 1) * TD], obp)
                if g == GRP - 1:
                    nc.gpsimd.dma_start(
                        out=preds_out[:, (b - GRP + 1) * TD:(b + 1) * TD], in_=obs)

    return nc


def _prep_inputs(x, eps, W1, b1, g1, beta1, W2, b2, g2, beta2, W3, b3,
                 Wih, Whh, bih, bhh, Wf, bf):
    x2 = np.ascontiguousarray(x[:, 0, :])                   # [B, 256]
    xin = np.ascontiguousarray(x2[:, :IN])                  # [B, 128]
    wih = np.ascontiguousarray(Wih[:, 0])                   # [3L]
    bb = bih + bhh
    common = {
        "w1t": np.ascontiguousarray(W1.T),
        "w2t": np.ascontiguousarray(W2.T),
        "w3t": np.ascontiguousarray(W3.T),
        "whht": np.ascontiguousarray(Whh.T),
        "wft": np.ascontiguousarray(Wf.T),
        "b1c": b1.reshape(H1, 1).copy(),
        "g1c": g1.reshape(H1, 1).copy(),
        "be1c": beta1.reshape(H1, 1).copy(),
        "b2c": b2.reshape(H1, 1).copy(),
        "g2c": g2.reshape(H1, 1).copy(),
        "be2c": beta2.reshape(H1, 1).copy(),
        "b3mu": b3[:L].reshape(L, 1).copy(),
        "b3lv": b3[L:].reshape(L, 1).copy(),
        "b3lvh": (0.5 * b3[L:]).reshape(L, 1).copy(),
        "wihc": np.ascontiguousarray(wih.reshape(3, L).T),
        "bbc": np.ascontiguousarray(
            np.stack([bb[:L], bb[L:2 * L], bhh[2 * L:]], axis=1)),
        "bihn": bih[2 * L:].reshape(L, 1).copy(),
    }
    in_maps = []
    for c in range(NC):
        m = dict(common)
        m["xinT"] = np.ascontiguousarray(np.roll(xin, -c * BS, axis=0).T)
        m["epsT"] = np.ascontiguousarray(np.roll(eps, -c * BS, axis=0).T)
        m["xs"] = np.ascontiguousarray(
            x2[c * BS:(c + 1) * BS, :TD]).reshape(1, BS * TD)
        in_maps.append({k: np.asarray(v, np.float32) for k, v in m.items()})
    return in_maps


def run(trace=False, **inputs):
    inputs = {k: np.asarray(v, np.float32) for k, v in inputs.items()}
    in_maps = _prep_inputs(**inputs)
    if "nc" not in _cache:
        _cache["nc"] = _build_bass()
    res = bass_utils.run_bass_kernel_spmd(
        _cache["nc"], in_maps, core_ids=list(range(NC)), trace=trace)
    outs = res.results
    bf = inputs["bf"]
    preds = np.concatenate(
        [outs[c]["preds"].reshape(2, BS, TD).transpose(1, 0, 2) for c in range(NC)],
        axis=0)
    preds = preds + bf.reshape(1, 2, 1)
    predictions = preds[:, None]                            # [B, 1, 2, TD]
    klp = outs[0]["klp"].reshape(L)
    kl = np.float32(-0.5 * (L + float(klp.sum()) / B))
    z = np.ascontiguousarray(outs[0]["zt"].T)               # [B, L]
    return (predictions, kl, z), res


def kernel(**inputs):
    out, _ = run(trace=False, **inputs)
    return out
